# revision 31
# baseline (speedup 1.0000x reference)
"""Trainium2 Bass kernel for nn_Attention_82540681494971.

Spatial self-attention block (unscaled):
  qkv = conv1x1(x);  s = q^T k  [n x n] per (b,head);  attn = softmax(s, axis=-1)
  out[d,m] = sum_n v[d,n] attn[n,m];  y = conv1x1(out)

Shapes: B=4, C=64, H=W=64 -> n=4096 tokens, HEAD=4, d=16.
Sharding: core c handles batch c//2, heads (0,1) if c%2==0 else (2,3).
Host sums the two per-core partial projections per batch and adds bias.

v2 design (ACT-engine-bound, ~249us exp floor):
 - Scores via 3-term bf16 split (K=48): s = [qh;ql;qh]^T [kh;kh;kl],
   fp32-grade. Two concurrent PE row groups (partition bases 0 / 64).
 - PSUM layout: scores ring = 5 banks (10 x 512-col granules), AV = 2
   banks, 1 bank for woven v / head-1 qk matmuls.
 - exp on ACT reads ring spans (up to 4096 wide, ~1.6 activates/tile
   amortizing the 352-cyc startup), writes bf16 E to SBUF. No accum.
 - rowsum = DVE reduce over bf16 E; rinv = DVE reciprocal; vts =
   v*rinv on GPSIMD (bf16).
 - AV: per (head, mc-chunk) ONE 32-matmul PSUM accumulation chain,
   4 chunks concurrent in 32-col strips of one [112,1024] tile.
 - Projection tail in bf16 (wp, out_h bf16; fp32 psum accum).
"""

import os
import numpy as np
from contextlib import ExitStack

DEBUG = os.environ.get("KDBG") == "1"

import concourse.bass as bass
import concourse.mybir as mybir
import concourse.tile as tile
from concourse import bacc
from concourse.bass import ts, ds
from concourse.bass_utils import run_bass_kernel_spmd

F32 = mybir.dt.float32
BF16 = mybir.dt.bfloat16
AF = mybir.ActivationFunctionType

B, C, HEAD, D = 4, 64, 4, 16
N = 4096          # tokens = H*W
NT = 128          # n-tile (partition) size
NTILES = N // NT  # 32
GR = 512          # score matmul moving width (fp32-safe granule)
GPT = N // GR     # granules per tile = 8
SCH = (1536, 1536, 1024)  # score chunk tiles (ping-pong pool, 6 banks)
MC = 512          # AV chunk width
RAMP = 12         # tiles with woven v/h1-qkv work (prologue psum pool open)
AV_LAG = 3        # steady-state AV emission lag (starts at RAMP, catches up)
E_BUFS = 14       # e-tile window
VTS_BUFS = 16


def _body(tc, y, x1, wq, wk, wv, wp0, wp1, dbg=None):
    nc = tc.nc
    ctx = ExitStack()
    with ctx:
        pp = ctx.enter_context(tc.tile_pool(name="persist", bufs=1))
        cp = ctx.enter_context(tc.tile_pool(name="consts", bufs=1))

        # ---- constants (wk first: gates the k matmuls) ----
        wq_t = cp.tile([C + 1, 2 * D], F32)
        wk_t = cp.tile([C + 1, 2 * D], F32)
        wv_t = cp.tile([C + 1, 2 * D], F32)
        wp0_t = cp.tile([D, C], BF16)
        wp1_t = cp.tile([D, C], BF16)
        nc.sync.dma_start(wk_t[:], wk[:])

        # ---- persistent SBUF ----
        # stacked q/k for scores: rows 0-47 = [hi; hi; lo] (q) or
        # [hi; lo; hi] (k); rows 64-111 = replica for PE row-group 2.
        qsp = pp.tile([112, 2 * N], BF16)
        ksp = pp.tile([112, 2 * N], BF16)
        # x in two tiles so early matmuls only depend on the first DMA
        x1a = pp.tile([C + 1, N // 2], F32)
        x1b = pp.tile([C + 1, N // 2], F32)

        def xsl(c):
            """x tile + col slice for 512-col chunk c."""
            t = x1a if c < 4 else x1b
            return t[:, ts(c % 4, GR)]

        def xnt(nt):
            """x tile + col slice for 128-col n-tile nt."""
            t = x1a if nt < 16 else x1b
            return t[:, ts(nt % 16, NT)]
        vT_sb = pp.tile([NT, NTILES * 2 * D], F32)   # [128, 1024] (h0|h1 per nt)
        out_h = [pp.tile([D, N], BF16, name=f"outh{h}") for h in range(2)]

        # ---- pools live for the whole kernel ----
        sp = ctx.enter_context(tc.tile_pool(name="sp", bufs=2, space="PSUM"))
        ep = ctx.enter_context(tc.tile_pool(name="ep", bufs=E_BUFS))
        vp = ctx.enter_context(tc.tile_pool(name="vp", bufs=VTS_BUFS))
        rp = ctx.enter_context(tc.tile_pool(name="rp", bufs=6))

        # =================================================================
        # Prologue: x load; k(h0), q(h0) -> hi/lo split -> stacked SBUF.
        # Uses a 3-bank psum pool (the banks later used by AV + vh1).
        #
        # Row layouts (block i of q pairs with block i of k):
        #   q = [hi@0, hi@16, lo@32],  k = [hi@0, lo@16, hi@32]
        #   -> s = qh*kh + qh*kl + ql*kh  (drops lo*lo)
        # Engines may only address partition bases 0/32/64/96, so rows
        # 16-31 and the 64-replica are filled by SBUF->SBUF DMA.
        # =================================================================
        nc.sync.dma_start(x1a[:], x1[:, 0:N // 2])
        nc.sync.dma_start(wq_t[:], wq[:])
        nc.sync.dma_start(x1b[:], x1[:, N // 2:N])
        nc.sync.dma_start(wv_t[:], wv[:])

        stp = ctx.enter_context(tc.tile_pool(name="stp", bufs=2))
        klo_pair = [None]

        def qk_split(which, dst, pk, c, h, pair=2):
            """Evacuate one [16, GR] psum chunk of q/k (hi cast on ACT for
            the prologue / DVE mid-phase, lo sub on DVE); on the last chunk
            of each `pair`-group, emit the fused dup/replica DMAs."""
            sl = ds(h * N + c * GR, GR)
            if h == 0:
                nc.scalar.copy(dst[ds(0, D), sl], pk[:])       # hi -> 0-15
            else:
                nc.vector.tensor_copy(dst[ds(0, D), sl], pk[:])
            if which == "q":
                nc.vector.tensor_sub(dst[ds(32, D), sl], pk[:],
                                     dst[ds(0, D), sl])        # lo -> 32-47
            else:
                if c % pair == 0:
                    klo_pair[0] = stp.tile([D, pair * GR], BF16, tag="lo",
                                           name=f"lo{h}_{c}")
                nc.vector.tensor_sub(klo_pair[0][:, ts(c % pair, GR)],
                                     pk[:], dst[ds(0, D), sl])
            if c % pair == pair - 1:
                c0 = c - (pair - 1)
                gsl = ds(h * N + c0 * GR, pair * GR)
                if which == "q":   # dup hi -> 16-31
                    nc.sync.dma_start(dst[ds(16, D), gsl], dst[ds(0, D), gsl])
                else:              # lo -> 16-31, dup hi -> 32-47
                    nc.sync.dma_start(dst[ds(16, D), gsl], klo_pair[0][:])
                    nc.sync.dma_start(dst[ds(32, D), gsl], dst[ds(0, D), gsl])
                nc.sync.dma_start(dst[ds(64, 48), gsl], dst[ds(0, 48), gsl])

        # prologue psum pool: stays open through the ramp tiles (v + h1
        # qk matmuls weave through it), closed before the AV pool opens.
        prol_cm = tc.tile_pool(name="prol", bufs=2, space="PSUM")
        prol = prol_cm.__enter__()
        def mk_qk0(which, c, pair=2):
            def go():
                w_t = wq_t if which == "q" else wk_t
                dst = qsp if which == "q" else ksp
                pk = prol.tile([D, GR], F32, tag="pq", name=f"{which}0_{c}")
                nc.tensor.matmul(pk[:], w_t[:, ds(0, D)], xsl(c),
                                 start=True, stop=True)
                qk_split(which, dst, pk, c, 0, pair=pair)
            return go

        # critical prefix: k chunks 0-3, q chunk 0 (covers score tiles 0-3),
        # v tiles 0-1.  k4-7 + q1-7 weave into the first tiles' chunk slots.
        for c in range(4):
            mk_qk0("k", c)()
        mk_qk0("q", 0, pair=1)()
        for nt in range(2):
            psv = prol.tile([NT, 2 * D], F32, tag="pq", name=f"pv{nt}")
            nc.tensor.matmul(psv[:], xnt(nt), wv_t[:],
                             start=True, stop=True)
            nc.vector.tensor_copy(vT_sb[:, ts(nt, 2 * D)], psv[:])
        nc.sync.dma_start(wp0_t[:], wp0[:])
        nc.sync.dma_start(wp1_t[:], wp1[:])

        # =================================================================
        # Main phase: 64 global tiles (h0 t0..31, h1 t0..31).
        # Per tile, per score chunk (1536/1536/1024): matmuls into a
        # ping-pong psum tile, then one exp activate into the e tile.
        # Rowsum on ACT-accum (ramp + 1/5 of tiles) or DVE reduce.
        # Tiles 0..RAMP-1 also weave v + h1 qk matmuls (prologue psum
        # pool still open); AV chains start at RAMP and catch up to
        # AV_LAG, accumulating in PSUM across all 32 n-tiles per head.
        # =================================================================
        work = []
        qk_work = []
        rs_dummy = pp.tile([NT, N], BF16, name="rs_dummy")
        e_tiles = [None] * 64
        vts_tiles = [None] * 64
        av_tile = [None, None]

        def tile_body(gt):
            h, t = gt // NTILES, gt % NTILES
            e_t = ep.tile([NT, N], BF16, tag="e", name=f"e{gt}")
            e_tiles[gt] = e_t
            use_act = gt < RAMP or gt % 6 == 5
            rsp = rp.tile([NT, 4], F32, tag="rsp", name="rsp") \
                if use_act else None
            off = 0
            for ci, csz in enumerate(SCH):
                # weave deferred q/k matmuls BEFORE the chunk that reads
                # them (tile 0 chunk ci reads k chunks 3ci..3ci+2, so two
                # pops per slot there)
                if ci > 0 or gt > 0:
                    pops = 2 if gt == 0 else 1
                    while qk_work and pops > 0:
                        qk_work.pop(0)()
                        pops -= 1
                s_ps = sp.tile([NT, SCH[0]], F32, tag="sa",
                               name=f"s{gt}_{ci}")
                for i in range(csz // GR):
                    G = gt * GPT + off // GR + i
                    base = 64 * (G % 2)     # PE row group
                    nc.tensor.matmul(
                        s_ps[:, ts(i, GR)],
                        qsp[ds(base, 3 * D), ds(h * N + t * NT, NT)],
                        ksp[ds(base, 3 * D), ds(h * N + off + i * GR, GR)],
                        start=True, stop=True)
                nc.scalar.activation(
                    e_t[:, ds(off, csz)], s_ps[:, 0:csz], AF.Exp,
                    accum_out=rsp[:, ds(ci, 1)] if use_act else None)
                off += csz
            # ---- rowsum -> rinv -> vts ----
            rs = rp.tile([NT, 1], F32, tag="rs", name="rs")
            rinv = rp.tile([NT, 1], F32, tag="ri", name="rinv")
            if use_act:
                nc.vector.reduce_sum(rs[:], rsp[:, 0:len(SCH)],
                                     axis=mybir.AxisListType.X)
            else:
                nc.vector.reduce_sum(rs[:], e_t[:],
                                     axis=mybir.AxisListType.X)
            nc.vector.reciprocal(rinv[:], rs[:])
            vts = vp.tile([NT, D], BF16, tag="vts", name=f"vts{gt}")
            nc.gpsimd.tensor_scalar_mul(
                vts[:], vT_sb[:, ds(t * 2 * D + h * D, D)], rinv[:])
            vts_tiles[gt] = vts
            if DEBUG and gt == 0:
                nc.sync.dma_start(dbg["e0"][:], e_t[:])
                nc.sync.dma_start(dbg["rs0"][:], rs[:])

        # ---- ramp tiles: prologue psum pool still open for v/h1 ----
        def mk_v(nt):
            def go():
                psv = prol.tile([NT, 2 * D], F32, tag="pq", name=f"v{nt}")
                nc.tensor.matmul(psv[:], xnt(nt), wv_t[:],
                                 start=True, stop=True)
                nc.vector.tensor_copy(vT_sb[:, ts(nt, 2 * D)], psv[:])
            return go

        def mk_qk1(which, c):
            def go():
                w_t = wq_t if which == "q" else wk_t
                dst = qsp if which == "q" else ksp
                pk = prol.tile([D, GR], F32, tag="pq", name=f"{which}1_{c}")
                nc.tensor.matmul(pk[:], w_t[:, ds(D, D)], xsl(c),
                                 start=True, stop=True)
                qk_split(which, dst, pk, c, 1)
            return go

        for c in range(4, GPT):
            qk_work.append(mk_qk0("k", c))
        for c in range(1, GPT):
            qk_work.append(mk_qk0("q", c))
        for c in range(GPT):
            qk_work.append(mk_qk1("k", c))
        for c in range(GPT):
            qk_work.append(mk_qk1("q", c))
        for nt in range(2, NTILES):
            work.append(mk_v(nt))

        for gt in range(RAMP):
            tile_body(gt)
            budget = 3
            while work and budget > 0:
                work.pop(0)()
                budget -= 1
        while work or qk_work:
            (qk_work if qk_work else work).pop(0)()
        prol_cm.__exit__(None, None, None)   # release prologue psum banks

        with tc.tile_pool(name="avp", bufs=1, space="PSUM") as avp:
            def emit_av(gt):
                h, t = gt // NTILES, gt % NTILES
                if t == 0:
                    av_tile[h] = [
                        avp.tile([112, MC], F32, tag=f"av{i}",
                                 name=f"av{h}_{i}") for i in range(2)]
                avs = av_tile[h]
                for mc in range(N // MC):
                    av = avs[mc // 4]
                    nc.tensor.matmul(
                        av[ds(32 * (mc % 4), D), :], vts_tiles[gt][:],
                        e_tiles[gt][:, ts(mc, MC)],
                        start=(t == 0), stop=(t == NTILES - 1),
                        tile_position=(0, 32 * (mc % 4)))
                if t == NTILES - 1:
                    for mc in range(N // MC):
                        nc.vector.tensor_copy(
                            out_h[h][:, ts(mc, MC)],
                            avs[mc // 4][ds(32 * (mc % 4), D), :])

            av_next = 0
            for gt in range(RAMP, 64):
                tile_body(gt)
                n_av = 0
                while av_next <= gt - AV_LAG and n_av < 3:
                    emit_av(av_next)
                    av_next += 1
                    n_av += 1
            while av_next < 64:
                emit_av(av_next)
                av_next += 1
            if DEBUG:
                nc.sync.dma_start(dbg["qsp"][:], qsp[:])
                nc.sync.dma_start(dbg["ksp"][:], ksp[:])
                nc.sync.dma_start(dbg["vT"][:], vT_sb[:])
                nc.sync.dma_start(dbg["oh0"][:], out_h[0][:])
                nc.sync.dma_start(dbg["oh1"][:], out_h[1][:])

        # =================================================================
        # Tail: projection y = [wp0; wp1] @ [out_h0; out_h1] (bf16), then
        # evacuate + DMA out per chunk.
        # =================================================================
        PC = 512
        with (
            tc.tile_pool(name="projp", bufs=2, space="PSUM") as projp,
            tc.tile_pool(name="yx", bufs=3) as yx,
        ):
            for mc in range(N // PC):
                yp = projp.tile([C, PC], F32, tag="yp", name=f"yp{mc}")
                nc.tensor.matmul(yp[:], wp0_t[:], out_h[0][:, ts(mc, PC)],
                                 start=True, stop=False)
                nc.tensor.matmul(yp[:], wp1_t[:], out_h[1][:, ts(mc, PC)],
                                 start=False, stop=True)
                yc = yx.tile([C, PC], F32, tag="y", name=f"yc{mc}")
                if mc % 2 == 0:
                    nc.vector.tensor_copy(yc[:], yp[:])
                else:
                    nc.scalar.copy(yc[:], yp[:])
                nc.sync.dma_start(y[:, ts(mc, PC)], yc[:])


_PROGRAM = None


def _get_program():
    global _PROGRAM
    if _PROGRAM is None:
        nc = bacc.Bacc("TRN2", target_bir_lowering=False, debug=False,
                       num_devices=8)
        x1 = nc.dram_tensor("x1", [C + 1, N], F32, kind="ExternalInput").ap()
        wq = nc.dram_tensor("wq", [C + 1, 2 * D], F32, kind="ExternalInput").ap()
        wk = nc.dram_tensor("wk", [C + 1, 2 * D], F32, kind="ExternalInput").ap()
        wv = nc.dram_tensor("wv", [C + 1, 2 * D], F32, kind="ExternalInput").ap()
        wp0 = nc.dram_tensor("wp0", [D, C], BF16, kind="ExternalInput").ap()
        wp1 = nc.dram_tensor("wp1", [D, C], BF16, kind="ExternalInput").ap()
        y = nc.dram_tensor("y", [C, N], F32, kind="ExternalOutput").ap()
        dbg = None
        if DEBUG:
            dbg = {
                "e0": nc.dram_tensor("e0", [NT, N], BF16, kind="ExternalOutput").ap(),
                "rs0": nc.dram_tensor("rs0", [NT, 1], F32, kind="ExternalOutput").ap(),
                "qsp": nc.dram_tensor("qspd", [112, 2 * N], BF16, kind="ExternalOutput").ap(),
                "ksp": nc.dram_tensor("kspd", [112, 2 * N], BF16, kind="ExternalOutput").ap(),
                "vT": nc.dram_tensor("vTd", [NT, NTILES * 2 * D], F32, kind="ExternalOutput").ap(),
                "oh0": nc.dram_tensor("oh0", [D, N], BF16, kind="ExternalOutput").ap(),
                "oh1": nc.dram_tensor("oh1", [D, N], BF16, kind="ExternalOutput").ap(),
            }
        with tile.TileContext(nc) as tc:
            _body(tc, y, x1, wq, wk, wv, wp0, wp1, dbg)
        nc.compile()
        _PROGRAM = nc
    return _PROGRAM


def _make_in_maps(x, qkv_w, qkv_b, proj_w, proj_b=None):
    from ml_dtypes import bfloat16
    x = np.asarray(x, dtype=np.float32)
    qkv_w = np.asarray(qkv_w, dtype=np.float32)
    qkv_b = np.asarray(qkv_b, dtype=np.float32)
    proj_w = np.asarray(proj_w, dtype=np.float32)

    in_maps = []
    for core in range(8):
        b = core // 2
        h0 = 2 * (core % 2)
        heads = (h0, h0 + 1)
        x1 = np.concatenate(
            [x[b].reshape(C, N), np.ones((1, N), np.float32)], axis=0)

        def aug_qk(block):
            w = np.empty((C + 1, 2 * D), np.float32)
            for j, h in enumerate(heads):
                rows = slice(block * C + h * D, block * C + (h + 1) * D)
                w[:C, j * D:(j + 1) * D] = qkv_w[rows, :].T
                w[C, j * D:(j + 1) * D] = qkv_b[rows]
            return w

        wp_parts = [
            np.ascontiguousarray(proj_w[:, h * D:(h + 1) * D].T).astype(bfloat16)
            for h in heads
        ]

        in_maps.append({
            "x1": np.ascontiguousarray(x1),
            "wq": aug_qk(0),
            "wk": aug_qk(1),
            "wv": aug_qk(2),
            "wp0": wp_parts[0],
            "wp1": wp_parts[1],
        })
    return in_maps


def run_cores(inputs, **kw):
    """Compile+run on the 8 cores; returns BassKernelResults."""
    nc = _get_program()
    in_maps = _make_in_maps(**inputs)
    return run_bass_kernel_spmd(nc, in_maps, list(range(8)), **kw)


def kernel(x, qkv_w, qkv_b, proj_w, proj_b):
    res = run_cores(dict(x=x, qkv_w=qkv_w, qkv_b=qkv_b,
                         proj_w=proj_w, proj_b=proj_b))
    proj_b = np.asarray(proj_b, dtype=np.float32)
    parts = [r["y"] for r in res.results]
    out = np.empty((B, C, N), np.float32)
    for b in range(B):
        out[b] = parts[2 * b] + parts[2 * b + 1] + proj_b[:, None]
    return out.reshape(B, C, 64, 64)


if __name__ == "__main__":
    _get_program()
    print("program built OK")
